# revision 1
# baseline (speedup 1.0000x reference)
"""GCN 2-layer encoder on 8 Trainium2 NeuronCores (Bass/Tile).

Strategy (graph partition by destination node):
  - nodes padded to NPAD and dst-sharded: core c owns rows [c*NPC, (c+1)*NPC)
  - per layer, every core holds the FULL projection table P = H @ W
    ([NPAD, 128] rows) in its local HBM:
      layer 1: each core computes P1 = x @ W1 itself (x replicated input)
      layer 2: each core computes its shard of P2 = out1 @ W2; one AllGather
      per superblock (chunked, overlapped with layer-1 work) assembles it
  - aggregation out[dst] = sum_e norm_e * P[src_e] (self-loops included as
    edges) runs per 128-dst block: dma_gather fetches P rows for the block's
    edges (int16 indices -> 4 src chunks of 25088 rows), one DVE
    tensor_scalar builds a norm-scaled one-hot [128 edges x 128 slots], and
    PE accumulates G_tile.T @ onehot into a PSUM bank, yielding the
    TRANSPOSED block [feat, slot]. ACT applies bias+ReLU out of PSUM.
  - P tables use a PERMUTED row layout (superblock-major) so that the
    per-superblock AllGather outputs are contiguous; the host permutes x
    and the gather indices to match. Final output is written transposed;
    the host transposes back.
"""

import sys

import numpy as np

sys.path.insert(0, "/opt/trn_rl_repo")

from contextlib import ExitStack
from dataclasses import dataclass, field


# ---------------------------------------------------------------- config

@dataclass
class Cfg:
    n: int = 100000          # real nodes
    feat: int = 128
    ncores: int = 8
    blk: int = 128           # dst block size (= PE tile)
    nblk: int = 98           # dst blocks per core
    chunk: int = 25088       # src chunk rows (int16 gather index limit)
    nch: int = 4
    sbb: int = 7             # dst blocks per superblock (gather granularity)
    maxg: int = 16384        # max indices per dma_gather call
    agg_bf16: bool = False   # gather tables + one-hot in bf16
    ablate: str = ""         # perf probes: "dma" (no DVE/PE), "dve" (no PE)
    ohb: int = 1             # one-hot batch (tiles per DVE op pair)
    gq: int = 1              # SWDGE queues to rotate gathers over (1..4)

    @property
    def npc(self):           # nodes per core
        return self.blk * self.nblk

    @property
    def npad(self):
        return self.npc * self.ncores

    @property
    def grp(self):           # rows per (core, superblock) group
        return self.sbb * self.blk

    def __post_init__(self):
        assert self.chunk * self.nch == self.npad
        assert self.chunk <= 32768
        assert self.nblk % self.sbb == 0

    @property
    def nsb(self):
        return self.nblk // self.sbb


CFG = Cfg()


# ---------------------------------------------------------------- host prep

@dataclass
class CallInfo:
    sb: int
    c: int
    L: int                   # padded edges in this gather call (mult of 128)
    icol: int                # column base into eidx [128, .] (int16, /16)
    scol: int                # column base into eslot/enorm [128, .] (/128)
    # list of (b_global, local_col0, T) for blocks in this call
    blocks: list = field(default_factory=list)


@dataclass
class Plan:
    calls: list              # list[CallInfo] in schedule order
    tot_tiles: int
    blk_tiles: np.ndarray    # [nblk, nch] tile counts (shared by all cores)


def build_plan(Ttab, cfg: Cfg):
    calls = []
    col = 0
    for sb in range(cfg.nsb):
        for c in range(cfg.nch):
            ci = CallInfo(sb=sb, c=c, L=0, icol=col * 8, scol=col)
            lcol = 0
            for b in range(sb * cfg.sbb, (sb + 1) * cfg.sbb):
                T = int(Ttab[b, c])
                if T:
                    ci.blocks.append((b, lcol, T))
                lcol += T
            ci.L = lcol * cfg.blk
            col += lcol
            calls.append(ci)
    return Plan(calls=calls, tot_tiles=col, blk_tiles=Ttab)


def node_perm(cfg: Cfg):
    """v -> permuted table row: superblock-group-major layout.

    row'(v) = (g * ncores + c) * grp + r  where c = v // npc,
    g = (v % npc) // grp, r = v % grp.
    """
    v = np.arange(cfg.npad, dtype=np.int64)
    c = v // cfg.npc
    g = (v % cfg.npc) // cfg.grp
    r = v % cfg.grp
    return (g * cfg.ncores + c) * cfg.grp + r


def preprocess(edge_index, x, W1, b1, W2, b2, cfg: Cfg):
    """Returns (plan, in_maps list per core)."""
    n, f = cfg.n, cfg.feat
    src = np.asarray(edge_index[0], dtype=np.int64)
    dst = np.asarray(edge_index[1], dtype=np.int64)

    deg = (np.bincount(dst, minlength=n) + 1).astype(np.float64)
    dinv = (1.0 / np.sqrt(deg)).astype(np.float32)

    loops = np.arange(n, dtype=np.int64)
    s_all = np.concatenate([src, loops])
    d_all = np.concatenate([dst, loops])
    norm = (dinv[s_all] * dinv[d_all]).astype(np.float32)

    perm = node_perm(cfg)                  # v -> table row
    srow = perm[s_all]                     # permuted src rows

    core = d_all // cfg.npc
    block = (d_all % cfg.npc) // cfg.blk
    slot = (d_all % cfg.blk).astype(np.float32)
    chunk = srow // cfg.chunk
    lsrc = (srow % cfg.chunk).astype(np.int16)

    gid = (core * cfg.nblk + block) * cfg.nch + chunk
    cnt = np.bincount(gid, minlength=cfg.ncores * cfg.nblk * cfg.nch)
    cnt = cnt.reshape(cfg.ncores, cfg.nblk, cfg.nch)
    Ttab = -(-cnt.max(axis=0) // cfg.blk)          # ceil, [nblk, nch]

    plan = build_plan(Ttab, cfg)
    LPAD = plan.tot_tiles * cfg.blk

    sched_key = np.empty((cfg.nblk, cfg.nch), dtype=np.int64)
    gbase = np.empty((cfg.nblk, cfg.nch), dtype=np.int64)
    k = 0
    colcum = 0
    for sb in range(cfg.nsb):
        for c in range(cfg.nch):
            for b in range(sb * cfg.sbb, (sb + 1) * cfg.sbb):
                sched_key[b, c] = k
                gbase[b, c] = colcum * cfg.blk
                colcum += Ttab[b, c]
                k += 1
    ngroups = k
    gbase_flat = np.empty(ngroups, dtype=np.int64)
    gbase_flat[sched_key.ravel()] = gbase.ravel()

    ekey = sched_key[block, chunk]                 # per-edge group id

    # xT in permuted column order: column row'(v) holds x[v]
    xT = np.zeros((f, cfg.npad), dtype=np.float32)
    xT[:, perm[:n]] = np.asarray(x, dtype=np.float32).T
    iota = np.broadcast_to(
        np.tile(np.arange(cfg.blk, dtype=np.float32), max(cfg.ohb, 1)),
        (f, max(cfg.ohb, 1) * cfg.blk)).copy()

    common = {
        "xT": xT,
        "W1": np.asarray(W1, dtype=np.float32),
        "W2": np.asarray(W2, dtype=np.float32),
        "b1": np.asarray(b1, dtype=np.float32).reshape(f, 1),
        "b2": np.asarray(b2, dtype=np.float32).reshape(f, 1),
        "iota": iota,
    }

    in_maps = []
    for cidx in range(cfg.ncores):
        sel = core == cidx
        ek = ekey[sel]
        order = np.argsort(ek, kind="stable")
        ek_s = ek[order]
        grp_counts = np.bincount(ek_s, minlength=ngroups)
        starts = np.concatenate([[0], np.cumsum(grp_counts)[:-1]])
        rank = np.arange(ek_s.size) - starts[ek_s]
        pos = gbase_flat[ek_s] + rank

        idx_flat = np.zeros(LPAD, dtype=np.int16)
        slot_flat = np.full(LPAD, -1.0, dtype=np.float32)
        norm_flat = np.zeros(LPAD, dtype=np.float32)
        idx_flat[pos] = lsrc[sel][order]
        slot_flat[pos] = slot[sel][order]
        norm_flat[pos] = norm[sel][order]

        idx16 = idx_flat.reshape(-1, 16).T                 # [16, LPAD/16]
        eidx = np.tile(idx16, (f // 16, 1))                # [128, LPAD/16]
        eslot = slot_flat.reshape(-1, cfg.blk).T.copy()    # [128, tiles]
        enorm = norm_flat.reshape(-1, cfg.blk).T.copy()

        in_maps.append(dict(common, eidx=eidx, eslot=eslot, enorm=enorm))

    return plan, in_maps


# ---------------------------------------------------------------- device

def build_module(plan: Plan, cfg: Cfg, phase_limit: str = "full", repeat: int = 1):
    import concourse.bacc as bacc
    import concourse.mybir as mybir
    import concourse.tile as tile

    f32 = mybir.dt.float32
    i16 = mybir.dt.int16
    gdt = mybir.dt.bfloat16 if cfg.agg_bf16 else f32
    F = cfg.feat
    LPAD = plan.tot_tiles * cfg.blk

    nc = bacc.Bacc(
        "TRN2",
        target_bir_lowering=False,
        debug=False,
        enable_asserts=False,
        num_devices=cfg.ncores,
        num_swdge_queues=cfg.gq,
    )

    xT_d = nc.dram_tensor("xT", [F, cfg.npad], f32, kind="ExternalInput").ap()
    W1_d = nc.dram_tensor("W1", [F, F], f32, kind="ExternalInput").ap()
    W2_d = nc.dram_tensor("W2", [F, F], f32, kind="ExternalInput").ap()
    b1_d = nc.dram_tensor("b1", [F, 1], f32, kind="ExternalInput").ap()
    b2_d = nc.dram_tensor("b2", [F, 1], f32, kind="ExternalInput").ap()
    OHW = max(cfg.ohb, 1) * cfg.blk
    iota_d = nc.dram_tensor("iota", [F, OHW], f32, kind="ExternalInput").ap()
    eidx_d = nc.dram_tensor("eidx", [F, LPAD // 16], i16, kind="ExternalInput").ap()
    eslot_d = nc.dram_tensor("eslot", [F, plan.tot_tiles], f32, kind="ExternalInput").ap()
    enorm_d = nc.dram_tensor("enorm", [F, plan.tot_tiles], f32, kind="ExternalInput").ap()
    out_d = nc.dram_tensor("outT", [F, cfg.npc], f32, kind="ExternalOutput").ap()

    eq, mul = mybir.AluOpType.is_equal, mybir.AluOpType.mult
    relu = mybir.ActivationFunctionType.Relu

    def block_last_c(b):
        for c in range(cfg.nch - 1, -1, -1):
            if plan.blk_tiles[b, c]:
                return c
        return -1

    with tile.TileContext(nc) as tc, ExitStack() as ctx:
        dram = ctx.enter_context(tc.tile_pool(name="dram", bufs=1, space="DRAM"))
        # per-chunk P1 tables so gathers can start before all of P1 is built
        P1c = [dram.tile([cfg.chunk, F], gdt, name=f"P1c{c}")
               for c in range(cfg.nch)]
        P2s = dram.tile([cfg.npc, F], gdt, name="P2s")
        P2f = dram.tile([cfg.npad, F], gdt, name="P2f")

        consts = ctx.enter_context(tc.tile_pool(name="consts", bufs=1))
        W1s = consts.tile([F, F], f32, name="W1s")
        W2s = consts.tile([F, F], f32, name="W2s")
        b1s = consts.tile([F, 1], f32, name="b1s")
        b2s = consts.tile([F, 1], f32, name="b2s")
        iotas = consts.tile([F, OHW], f32, name="iotas")
        nc.sync.dma_start(W1s[:], W1_d)
        nc.sync.dma_start(W2s[:], W2_d)
        nc.sync.dma_start(b1s[:], b1_d)
        nc.sync.dma_start(b2s[:], b2_d)
        nc.sync.dma_start(iotas[:], iota_d)

        big = ctx.enter_context(tc.tile_pool(name="big", bufs=1))
        out1T = big.tile([F, cfg.npc], f32, name="out1T")

        # perf-probe amplification: wrap the compute phases in a device loop
        rep_ctx = ExitStack()
        if repeat > 1:
            assert phase_limit == "AB", "repeat probe only for AB phase"
            rep_ctx.enter_context(tc.For_i(0, repeat, 1))

        # ---------------- phase A: P1 = x @ W1 (full, replicated) ----------
        CH = cfg.grp  # 896 columns per step; cfg.chunk % CH == 0
        assert cfg.chunk % CH == 0
        with (
            tc.tile_pool(name="xa", bufs=3) as xa,
            tc.tile_pool(name="pa", bufs=8, space="PSUM") as pa,
            tc.tile_pool(name="sa", bufs=3) as sa,
        ):
            per_chunk = cfg.chunk // CH
            PW = 512  # psum bank width; 4 matmul outputs per bank
            for i in range(cfg.npad // CH):
                ch_id, ch_off = divmod(i, per_chunk)
                xt = xa.tile([F, CH], f32, name="xt")
                nc.sync.dma_start(xt[:], xT_d[:, i * CH:(i + 1) * CH])
                st = sa.tile([F, CH], gdt, name="st")
                for j in range(0, CH, PW):
                    w = min(PW, CH - j)
                    ps = pa.tile([F, PW], f32, name="ps")
                    for k in range(0, w, cfg.blk):
                        nc.tensor.matmul(
                            ps[:, k:k + cfg.blk],
                            xt[:, j + k:j + k + cfg.blk], W1s[:],
                            start=True, stop=True,
                        )
                    nc.vector.tensor_copy(st[:, j:j + w], ps[:, :w])
                dst = P1c[ch_id][ch_off * CH:(ch_off + 1) * CH, :].rearrange(
                    "(k p) f -> p k f", p=F)
                nc.sync.dma_start(dst, st[:].rearrange("p (k f) -> p k f", f=F))

        # ---------------- aggregation (shared by both layers) --------------
        def aggregate(tables, writer, post_sb=None):
            aggregate.gqctr = getattr(aggregate, "gqctr", 0)
            with (
                tc.tile_pool(name="gp", bufs=3) as gp,
                tc.tile_pool(name="ip", bufs=2) as ip,
                tc.tile_pool(name="sp", bufs=2) as sp,
                tc.tile_pool(name="nppool", bufs=2) as npp,
                tc.tile_pool(name="ohp", bufs=4) as ohp,
                tc.tile_pool(name="aggp", bufs=cfg.sbb + 1, space="PSUM") as aggp,
            ):
                aggregate.psum_pool = aggp
                for sb in range(cfg.nsb):
                    pss = {}
                    started = {}
                    for b in range(sb * cfg.sbb, (sb + 1) * cfg.sbb):
                        if not cfg.ablate:
                            pss[b] = aggp.tile([F, cfg.blk], f32, name="aggps",
                                               tag="aggps")
                        started[b] = False
                    for c in range(cfg.nch):
                        if cfg.ablate == "pa":
                            continue
                        call = plan.calls[sb * cfg.nch + c]
                        L = call.L
                        if L == 0:
                            continue
                        gt = gp.tile([F, L], gdt, name="gt")
                        it = ip.tile([F, L // 16], i16, name="it")
                        nc.sync.dma_start(
                            it[:], eidx_d[:, call.icol:call.icol + L // 16])
                        st2 = sp.tile([F, L // cfg.blk], f32, name="st2")
                        nc.sync.dma_start(
                            st2[:], eslot_d[:, call.scol:call.scol + L // cfg.blk])
                        nt = npp.tile([F, L // cfg.blk], f32, name="nt")
                        nc.sync.dma_start(
                            nt[:], enorm_d[:, call.scol:call.scol + L // cfg.blk])
                        # split the gather into <= maxg-index calls
                        for off in range(0, L, cfg.maxg):
                            ln = min(cfg.maxg, L - off)
                            nc.gpsimd.dma_gather(
                                gt[:, off:off + ln].rearrange(
                                    "p (t f) -> p t f", f=F),
                                tables[c][:],
                                it[:, off // 16:(off + ln) // 16],
                                num_idxs=ln,
                                num_idxs_reg=ln,
                                elem_size=F,
                                single_packet=cfg.maxg <= 1024,
                                queue_num=aggregate.gqctr % cfg.gq,
                            )
                            aggregate.gqctr += 1
                        if cfg.ablate == "dma":
                            continue
                        # tile -> (block, t, last flag) map for this call
                        meta = {}
                        for (b, lcol0, T) in call.blocks:
                            last_c = block_last_c(b)
                            for t in range(T):
                                meta[lcol0 + t] = (
                                    b, c == last_c and t == T - 1)
                        ncols = L // cfg.blk
                        for base in range(0, ncols, cfg.ohb):
                            bw = min(cfg.ohb, ncols - base)
                            W = bw * cfg.blk
                            oh = ohp.tile([F, OHW], gdt, name="oh")
                            if cfg.ohb == 1:
                                nc.vector.tensor_scalar(
                                    oh[:, :W], iotas[:, :W],
                                    st2[:, base:base + 1],
                                    nt[:, base:base + 1],
                                    eq, mul,
                                )
                            else:
                                oh3 = oh[:, :W].rearrange(
                                    "p (b s) -> p b s", s=cfg.blk)
                                nc.vector.tensor_tensor(
                                    oh3, iotas[:, :W].rearrange(
                                        "p (b s) -> p b s", s=cfg.blk),
                                    st2[:, base:base + bw].to_broadcast(
                                        [F, bw, cfg.blk]),
                                    eq,
                                )
                                nc.vector.tensor_tensor(
                                    oh3, oh3,
                                    nt[:, base:base + bw].to_broadcast(
                                        [F, bw, cfg.blk]),
                                    mul,
                                )
                            if cfg.ablate == "dve":
                                continue
                            for k in range(bw):
                                lc = base + k
                                b, is_stop = meta[lc]
                                nc.tensor.matmul(
                                    pss[b][:],
                                    gt[:, lc * cfg.blk:(lc + 1) * cfg.blk],
                                    oh[:, k * cfg.blk:(k + 1) * cfg.blk],
                                    start=not started[b],
                                    stop=is_stop,
                                )
                                started[b] = True
                    if not cfg.ablate:
                        for b in range(sb * cfg.sbb, (sb + 1) * cfg.sbb):
                            writer(b, pss[b])
                        if post_sb is not None:
                            post_sb(sb)

        # ---------------- phase B: layer-1 aggregation ---------------------
        def w1_writer(b, ps):
            nc.scalar.activation(
                out1T[:, b * cfg.blk:(b + 1) * cfg.blk], ps[:],
                relu, bias=b1s[:, 0:1], scale=1.0,
            )

        # per-superblock: P2 shard rows + chunked AllGather (overlapped)
        b2pool = ctx.enter_context(tc.tile_pool(name="b2st", bufs=3))

        def post_sb_l1(sb):
            if phase_limit == "AB":
                return
            st3 = b2pool.tile([F, cfg.grp], gdt, name="st3")
            for k in range(cfg.sbb):
                b = sb * cfg.sbb + k
                ps2 = aggregate.psum_pool.tile([F, cfg.blk], f32, name="ps2",
                                               tag="aggps")
                nc.tensor.matmul(
                    ps2[:], out1T[:, b * cfg.blk:(b + 1) * cfg.blk], W2s[:],
                    start=True, stop=True,
                )
                nc.vector.tensor_copy(
                    st3[:, k * cfg.blk:(k + 1) * cfg.blk], ps2[:])
            rows = cfg.grp
            dst = P2s[sb * rows:(sb + 1) * rows, :].rearrange(
                "(k p) f -> p k f", p=F)
            nc.sync.dma_start(dst, st3[:].rearrange("p (k f) -> p k f", f=F))
            # chunked AllGather: group sb rows -> P2f[sb*grp*ncores ...]
            nc.gpsimd.collective_compute(
                "AllGather",
                mybir.AluOpType.bypass,
                replica_groups=[list(range(cfg.ncores))],
                ins=[P2s[sb * rows:(sb + 1) * rows, :].opt()],
                outs=[P2f[sb * rows * cfg.ncores:(sb + 1) * rows * cfg.ncores,
                          :].opt()],
            )

        aggregate(P1c, w1_writer, post_sb=post_sb_l1)

        rep_ctx.close()

        if phase_limit == "AB":
            if cfg.ablate:
                nc.sync.dma_start(out_d[:, :cfg.blk], iotas[:, :cfg.blk])
            else:
                nc.sync.dma_start(out_d[:], out1T[:])

        # ---------------- phase D: layer-2 aggregation ---------------------
        if phase_limit == "full":
            P2fc = [P2f[c * cfg.chunk:(c + 1) * cfg.chunk, :]
                    for c in range(cfg.nch)]
            with tc.tile_pool(name="op", bufs=4) as op:
                def w2_writer(b, ps):
                    o = op.tile([F, cfg.blk], f32, name="o")
                    nc.scalar.activation(
                        o[:], ps[:], relu, bias=b2s[:, 0:1], scale=1.0)
                    nc.sync.dma_start(
                        out_d[:, b * cfg.blk:(b + 1) * cfg.blk], o[:])

                aggregate(P2fc, w2_writer)

    nc.compile()
    return nc


# ---------------------------------------------------------------- entry

def run(inputs, cfg: Cfg = CFG, trace=False, phase_limit="full"):
    from concourse import bass_utils

    plan, in_maps = preprocess(
        inputs["edge_index"], inputs["x"],
        inputs["W1"], inputs["b1"], inputs["W2"], inputs["b2"], cfg,
    )
    nc = build_module(plan, cfg, phase_limit=phase_limit)
    res = bass_utils.run_bass_kernel_spmd(
        nc, in_maps, core_ids=list(range(cfg.ncores)), trace=trace,
    )
    shards = [res.results[c]["outT"] for c in range(cfg.ncores)]
    out = np.concatenate([s.T for s in shards], axis=0)[:cfg.n]
    return np.ascontiguousarray(out), res


def kernel(**inputs) -> np.ndarray:
    out, _ = run(inputs)
    return out

